# revision 1
# baseline (speedup 1.0000x reference)
"""Trainium2 Bass kernel for nn_DilatedConv (dense_cnn).

Math: the torch in-place dilated-conv loop is the affine recurrence
    s[t+1] = A @ s[t] + c[t],   A = weight[:, :, 0],  c[t] = W1 @ x[:, :, t + n_dil]
over n_steps = N - ((n_filt-1)*n_dil + 1) = 7935 transitions, with
s[0] = x[:, :, 0]; outputs overwrite x[:, :, 1 : 1+n_steps].

Parallelization: data-parallel over batch (16 -> 2 per core on 8 cores);
per core a 4-level radix-8 blocked scan over T=8192 padded transitions
(8 x 8 x 8 x 16).  Matrix powers (A^k, transposed into 128x128 lhsT tiles)
are precomputed on the host in float64.  All device matmuls run as
float32r (bf16-pair split; measured identical numerics to the fp32 PE
path on TRN2, ~1.7e-4 per-op rel err, 4x faster).

Device phases (per core, everything resident in SBUF):
  C:      c[t] = W1 @ x[t+n_dil]  (big batched matmul, x streamed from HBM)
  up1:    per-8-block zero-init finals l1[g] = sum_j A^{7-j} c[8g+j]
          (stacked-powers matmul, no sequential chain)
  up2/3:  radix-8 reductions of l1 -> l2 -> l3 (short sequential scans)
  chain4: 15-step sequential scan over superblocks (A^512)
  down3/2: expand superblock inits back down (A^64, A^8 steps)
  down1:  7 steps s[8g+i] = A s[8g+i-1] + c[8g+i-1]; each step's output
          overwrites the just-consumed c slot => slot[p] = s[p+1], so the
          final output DMA is fully contiguous.
The "+ c" additions ride the PE as a third accumulating identity matmul,
so every PSUM->SBUF transfer is a pure copy (split across DVE and ACT).
"""

import numpy as np

# ---------------- problem constants (hardcoded per spec) ----------------
B_FULL = 16
C = 256
N = 8192
N_DIL = 256
N_FILT = 2
N_CORES = 8
B_LOC = B_FULL // N_CORES          # 2

FILTER = (N_FILT - 1) * N_DIL + 1  # 257
N_STEPS = N - FILTER               # 7935
TMAX = N - N_DIL                   # 7936 = n_steps + 1 (c[t] real for t < TMAX)

REAL_CFG = dict(N=N, T=8192, M1=8, M2=8, M3=8)


def _host_pack(weight_f32, cfg):
    """Pack all lhsT 128x128 tiles into one (NT,128,128) float32 array.

    matmul(out, lhsT, rhs) computes lhsT.T @ rhs, so for out = Mat @ v the
    (kc, mc) tile is Mat.T[128kc:128(kc+1), 128mc:128(mc+1)].
    Layout (tile index):
      0..3                 W1T        [2*kc+mc]
      4+4j+2kc+mc          (A^(M1-1-j)).T   j=0..M1-1
      base2 = 4+4*M1 ..+3  A8T   (A^M1)
      base3 ..+3           A64T  (A^(M1*M2))
      base4 ..+3           A512T (A^(M1*M2*M3))
      NT-2                 identity
      NT-1                 zeros
    """
    M1, M2, M3 = cfg["M1"], cfg["M2"], cfg["M3"]
    A = weight_f32[:, :, 0].astype(np.float64)
    W1 = weight_f32[:, :, 1].astype(np.float64)

    def tiles(mat):
        mt = mat.T.astype(np.float32)
        return [mt[128 * kc:128 * (kc + 1), 128 * mc:128 * (mc + 1)]
                for kc in range(2) for mc in range(2)]

    pack = []
    pack += tiles(W1)
    Apow = [np.linalg.matrix_power(A, p) for p in range(max(M1, 2))]
    for j in range(M1):
        pack += tiles(Apow[M1 - 1 - j])
    A_m1 = np.linalg.matrix_power(A, M1)
    A_m12 = np.linalg.matrix_power(A, M1 * M2)
    A_m123 = np.linalg.matrix_power(A, M1 * M2 * M3)
    pack += tiles(A_m1)
    pack += tiles(A_m12)
    pack += tiles(A_m123)
    pack.append(np.eye(128, dtype=np.float32))
    pack.append(np.zeros((128, 128), dtype=np.float32))
    return np.stack(pack, axis=0)


def _build_program(cfg):
    """Build + bacc-compile the per-core Bass program. Returns nc."""
    import concourse.bacc as bacc
    import concourse.tile as tile
    from concourse import mybir

    f32r = mybir.dt.float32r
    f32 = mybir.dt.float32

    Nl = cfg["N"]
    T = cfg["T"]
    M1, M2, M3 = cfg["M1"], cfg["M2"], cfg["M3"]
    G1 = T // M1
    G2 = G1 // M2
    G3 = G2 // M3
    n_steps = Nl - FILTER
    tmax = Nl - N_DIL
    assert T >= n_steps + 1
    NT = 4 + 4 * M1 + 12 + 2
    W1T = lambda kc, mc: 2 * kc + mc
    UP1 = lambda j, kc, mc: 4 + 4 * j + 2 * kc + mc
    A8T = lambda kc, mc: 4 + 4 * M1 + 2 * kc + mc
    A64T = lambda kc, mc: 4 + 4 * M1 + 4 + 2 * kc + mc
    A512T = lambda kc, mc: 4 + 4 * M1 + 8 + 2 * kc + mc
    AT = lambda kc, mc: UP1(M1 - 2, kc, mc)   # A^1 tiles
    IDENT = NT - 2
    ZERO = NT - 1

    nc = bacc.Bacc("TRN2", target_bir_lowering=False, debug=False,
                   num_devices=N_CORES)
    x_in = nc.dram_tensor("x", [B_LOC, C, Nl], f32r, kind="ExternalInput").ap()
    wp_in = nc.dram_tensor("wpack", [NT, 128, 128], f32r,
                           kind="ExternalInput").ap()
    out = nc.dram_tensor("out", [B_LOC, C, Nl], f32r, kind="ExternalOutput").ap()

    CHUNK = 512  # free-dim chunk for big matmuls (one PSUM bank of fp32)

    with tile.TileContext(nc) as tc:
        import contextlib
        with contextlib.ExitStack() as ctx:
            wpool = ctx.enter_context(tc.tile_pool(name="wpool", bufs=1))
            cpool = ctx.enter_context(tc.tile_pool(name="cpool", bufs=1))
            spool = ctx.enter_context(tc.tile_pool(name="spool", bufs=1))
            xwin = ctx.enter_context(tc.tile_pool(name="xwin", bufs=3))
            pspool = ctx.enter_context(tc.tile_pool(name="ps", bufs=8,
                                                    space="PSUM"))

            # ---- persistent tiles ----
            wpk = wpool.tile([128, NT * 128], f32r, tag="wpk", name="wpk")
            wt = lambda i: wpk[:, 128 * i:128 * (i + 1)]
            ctile = [cpool.tile([128, B_LOC * T], f32r, tag=f"c{mc}", name=f"c{mc}")
                     for mc in range(2)]
            l1 = [spool.tile([128, B_LOC * G1], f32r, tag=f"l1_{mc}", name=f"l1_{mc}")
                  for mc in range(2)]
            u1 = [spool.tile([128, B_LOC * G1], f32r, tag=f"u1_{mc}", name=f"u1_{mc}")
                  for mc in range(2)]
            l2 = [spool.tile([128, B_LOC * G2], f32r, tag=f"l2_{mc}", name=f"l2_{mc}")
                  for mc in range(2)]
            u3 = [spool.tile([128, B_LOC * G2], f32r, tag=f"u3_{mc}", name=f"u3_{mc}")
                  for mc in range(2)]
            l3 = [spool.tile([128, B_LOC * G3], f32r, tag=f"l3_{mc}", name=f"l3_{mc}")
                  for mc in range(2)]
            u4 = [spool.tile([128, B_LOC * G3], f32r, tag=f"u4_{mc}", name=f"u4_{mc}")
                  for mc in range(2)]
            s0 = [spool.tile([128, B_LOC], f32r, tag=f"s0_{mc}", name=f"s0_{mc}")
                  for mc in range(2)]

            # round-robin copy engines for PSUM->SBUF (only DVE + ACT touch PSUM)
            _cp = [0]

            def copy_ps(dst, src):
                if _cp[0] % 2 == 0:
                    nc.vector.tensor_copy(dst, src)
                else:
                    nc.scalar.copy(dst, src)
                _cp[0] += 1

            _cps = [0]

            def copy_sb(dst, src):
                e = _cps[0] % 3
                if e == 0:
                    nc.vector.tensor_copy(dst, src)
                elif e == 1:
                    nc.gpsimd.tensor_copy(dst, src)
                else:
                    nc.scalar.copy(dst, src)
                _cps[0] += 1

            # ---- phase 0: loads + passthrough edges ----
            # W1T tiles first in their own DMA so stage C starts immediately
            # (deps are subregion-granular); the big rest follows.
            nc.sync.dma_start(
                wpk[:, 0:4 * 128].rearrange("p (t f) -> p t f", t=4),
                wp_in[0:4].rearrange("t p f -> p t f"))
            nc.sync.dma_start(
                wpk[:, 4 * 128:].rearrange("p (t f) -> p t f", t=NT - 4),
                wp_in[4:].rearrange("t p f -> p t f"))
            for mc in range(2):
                for b in range(B_LOC):
                    nc.sync.dma_start(
                        s0[mc][:, b:b + 1],
                        x_in[b, 128 * mc:128 * (mc + 1), 0:1])
            # untouched output regions: tail straight DRAM->DRAM, col 0 via s0
            nc.sync.dma_start(out[:, :, tmax:Nl], x_in[:, :, tmax:Nl])
            for mc in range(2):
                for b in range(B_LOC):
                    nc.sync.dma_start(
                        out[b, 128 * mc:128 * (mc + 1), 0:1],
                        s0[mc][:, b:b + 1])

            # ---- phase C: c[t] = W1 @ x[:, t + N_DIL], t in [0, tmax); 0 beyond ----
            n_tq = (T + CHUNK - 1) // CHUNK
            for b in range(B_LOC):
                for tq in range(n_tq):
                    t0 = tq * CHUNK
                    w = min(CHUNK, tmax - t0)
                    if w <= 0:
                        break
                    xw = [xwin.tile([128, CHUNK], f32r, tag=f"xw{kc}", name=f"xw{kc}")
                          for kc in range(2)]
                    for kc in range(2):
                        nc.sync.dma_start(
                            xw[kc][:, :w],
                            x_in[b, 128 * kc:128 * (kc + 1),
                                 N_DIL + t0:N_DIL + t0 + w])
                    for mc in range(2):
                        ps = pspool.tile([128, CHUNK], f32, tag="ps", name="ps")
                        for kc in range(2):
                            nc.tensor.matmul(ps[:, :w], wt(W1T(kc, mc)),
                                             xw[kc][:, :w],
                                             start=(kc == 0), stop=(kc == 1))
                        copy_ps(ctile[mc][:, b * T + t0:b * T + t0 + w],
                                ps[:, :w])
                # zero pad c[t] for t in [tmax, T)
                npad = T - tmax
                for mc in range(2):
                    off = b * T + tmax
                    done = 0
                    while done < npad:
                        wz = min(128, npad - done)
                        copy_sb(ctile[mc][:, off + done:off + done + wz],
                                wt(ZERO)[:, :wz])
                        done += wz

            # ---- phase up1: l1[g] = sum_j A^{M1-1-j} c[M1*g + j] ----
            for b in range(B_LOC):
                for g0 in range(0, G1, CHUNK):
                    gw = min(CHUNK, G1 - g0)
                    for mc in range(2):
                        ps = pspool.tile([128, CHUNK], f32, tag="ps", name="ps")
                        for j in range(M1):
                            for kc in range(2):
                                rhs = ctile[kc][:, b * T + g0 * M1 + j:
                                                b * T + (g0 + gw) * M1:M1]
                                nc.tensor.matmul(
                                    ps[:, :gw], wt(UP1(j, kc, mc)), rhs,
                                    start=(j == 0 and kc == 0),
                                    stop=(j == M1 - 1 and kc == 1))
                        copy_ps(l1[mc][:, b * G1 + g0:b * G1 + g0 + gw],
                                ps[:, :gw])

            # ---- up2: l2[h] = scan over l1[M2*h + r] with A8 ----
            for mc in range(2):
                copy_sb(l2[mc][:], l1[mc][:, 0::M2])
            for r in range(1, M2):
                pss = []
                for mc in range(2):
                    ps = pspool.tile([128, B_LOC * G2], f32, tag="ps", name="ps")
                    for kc in range(2):
                        nc.tensor.matmul(ps[:], wt(A8T(kc, mc)), l2[kc][:],
                                         start=(kc == 0), stop=(kc == 1))
                    pss.append(ps)
                # adds emitted after BOTH mc matmul groups: they must read
                # the pre-step l2 values (in-place full overwrite)
                for mc in range(2):
                    nc.vector.tensor_add(l2[mc][:], pss[mc][:],
                                         l1[mc][:, r::M2])

            # ---- up3: l3[q] = scan over l2[M3*q + r] with A64 ----
            for mc in range(2):
                copy_sb(l3[mc][:], l2[mc][:, 0::M3])
            for r in range(1, M3):
                pss = []
                for mc in range(2):
                    ps = pspool.tile([128, B_LOC * G3], f32, tag="ps", name="ps")
                    for kc in range(2):
                        nc.tensor.matmul(ps[:], wt(A64T(kc, mc)), l3[kc][:],
                                         start=(kc == 0), stop=(kc == 1))
                    pss.append(ps)
                for mc in range(2):
                    nc.vector.tensor_add(l3[mc][:], pss[mc][:],
                                         l2[mc][:, r::M3])

            # ---- chain4: u4[q+1] = A512 u4[q] + l3[q], u4[0] = s0 ----
            for mc in range(2):
                copy_sb(u4[mc][:, 0::G3], s0[mc][:])
            for q in range(G3 - 1):
                for mc in range(2):
                    ps = pspool.tile([128, B_LOC], f32, tag="ps", name="ps")
                    for kc in range(2):
                        nc.tensor.matmul(ps[:], wt(A512T(kc, mc)),
                                         u4[kc][:, q::G3],
                                         start=(kc == 0), stop=(kc == 1))
                    nc.vector.tensor_add(u4[mc][:, q + 1::G3], ps[:],
                                         l3[mc][:, q::G3])

            # ---- down3: u3[M3 q + r], init u4, transfer A64, add l2 ----
            for mc in range(2):
                copy_sb(u3[mc][:, 0::M3], u4[mc][:])
            for r in range(1, M3):
                for mc in range(2):
                    ps = pspool.tile([128, B_LOC * G3], f32, tag="ps", name="ps")
                    for kc in range(2):
                        nc.tensor.matmul(ps[:], wt(A64T(kc, mc)),
                                         u3[kc][:, r - 1::M3],
                                         start=(kc == 0), stop=(kc == 1))
                    nc.vector.tensor_add(u3[mc][:, r::M3], ps[:],
                                         l2[mc][:, r - 1::M3])

            # ---- down2: u1[M2 h + j], init u3, transfer A8, add l1 ----
            for mc in range(2):
                copy_sb(u1[mc][:, 0::M2], u3[mc][:])
            for j in range(1, M2):
                for mc in range(2):
                    ps = pspool.tile([128, B_LOC * G2], f32, tag="ps", name="ps")
                    for kc in range(2):
                        nc.tensor.matmul(ps[:], wt(A8T(kc, mc)),
                                         u1[kc][:, j - 1::M2],
                                         start=(kc == 0), stop=(kc == 1))
                    nc.vector.tensor_add(u1[mc][:, j::M2], ps[:],
                                         l1[mc][:, j - 1::M2])

            # ---- down1: slots: c[p] <- s[p+1], per (b, g-chunk) so each
            # chunk's output DMA overlaps the next chunk's scan ----
            for b in range(B_LOC):
                for g0 in range(0, G1, CHUNK):
                    gw = min(CHUNK, G1 - g0)
                    base = b * T + g0 * M1
                    for i in range(1, M1):
                        for mc in range(2):
                            ps = pspool.tile([128, CHUNK], f32, tag="ps", name="ps")
                            for kc in range(2):
                                if i == 1:
                                    rhs = u1[kc][:, b * G1 + g0:
                                                 b * G1 + g0 + gw]
                                else:
                                    rhs = ctile[kc][:, base + i - 2:
                                                    base + gw * M1:M1]
                                nc.tensor.matmul(ps[:, :gw], wt(AT(kc, mc)),
                                                 rhs, start=(kc == 0),
                                                 stop=False)
                            nc.tensor.matmul(
                                ps[:, :gw], wt(IDENT),
                                ctile[mc][:, base + i - 1:base + gw * M1:M1],
                                start=False, stop=True)
                            copy_ps(
                                ctile[mc][:, base + i - 1:base + gw * M1:M1],
                                ps[:, :gw])
                    # residue: slot[M1 g + M1-1] = s[M1 (g+1)] = u1[g+1];
                    # the very last slot (s[T]) is past n_steps -> skipped
                    ng = gw if g0 + gw < G1 else gw - 1
                    for mc in range(2):
                        copy_sb(
                            ctile[mc][:, base + M1 - 1:base + ng * M1:M1],
                            u1[mc][:, b * G1 + g0 + 1:b * G1 + g0 + ng + 1])
                    # this chunk's slots are final: DMA them out now
                    lo = g0 * M1
                    hi = min(g0 * M1 + gw * M1, n_steps)
                    for mc in range(2):
                        nc.sync.dma_start(
                            out[b, 128 * mc:128 * (mc + 1), 1 + lo:1 + hi],
                            ctile[mc][:, b * T + lo:b * T + hi])

    nc.compile()
    return nc


_CACHE = {}


def _get_program(cfg_key=None):
    cfg = REAL_CFG if cfg_key is None else cfg_key
    key = tuple(sorted(cfg.items()))
    if key not in _CACHE:
        _CACHE[key] = _build_program(cfg)
    return _CACHE[key]


LAST_RESULTS = None  # test harness reads exec_time_ns off this


def kernel(x, weight, n_dil):
    import os
    from concourse.bass_utils import run_bass_kernel_spmd
    global LAST_RESULTS

    x = np.asarray(x)
    weight = np.asarray(weight)
    assert int(n_dil) == N_DIL and x.shape == (B_FULL, C, N)
    nc = _get_program()
    wpack = _host_pack(weight.astype(np.float32), REAL_CFG)

    xs = x.astype(np.float32).reshape(N_CORES, B_LOC, C, N)
    in_maps = [{"x": xs[i], "wpack": wpack} for i in range(N_CORES)]
    trace = bool(os.environ.get("KERNEL_TRACE"))
    res = run_bass_kernel_spmd(nc, in_maps, list(range(N_CORES)), trace=trace)
    LAST_RESULTS = res
    out = np.concatenate([res.results[i]["out"] for i in range(N_CORES)],
                         axis=0)
    return out.astype(x.dtype, copy=False)

